# revision 28
# baseline (speedup 1.0000x reference)
"""MiniS4D Trainium2 kernel (8 NeuronCores, batch-data-parallel).

Math: the reference convolves u with the TIME-REVERSED S4D kernel
(keff[lag] = K[L-1-lag], built from growing exponentials 1/r). Stable
chunked decomposition used here (T=128, M=32 chunks):

  y[l] = intra + inter
  intra (lags 0..127): per-channel Toeplitz matmul with keff[0:128]
         (+ D folded into lag 0).
  inter (lags >= 128): y_inter[128m+j] = Re sum_n w_n r_n^{-j} *
         Htil_n[m],  Htil[m] = r^{L-1-128m} * PP[m-1],
         PP[m] = prefix-sum over chunks of e^m * Q[m],  e = r^128,
         Q[m] = sum_i r^i u[128m+i]   (all factors bounded, |r|<1).

Per core: 2 batch elements, all 512 channels; fp16 operands, fp32
accumulation. Output (16,1) assembled on host from per-core (1,2).
"""
import sys, os
sys.path.insert(0, "/opt/trn_rl_repo")
import numpy as np

import concourse.bass as bass
import concourse.tile as tile
from concourse import bacc, mybir
from concourse import bass_utils

F32 = mybir.dt.float32
F16 = mybir.dt.float16
AF = mybir.ActivationFunctionType
ALU = mybir.AluOpType

B, C, L, N = 16, 512, 4096, 8
T, M = 128, 32
NCORES, BL = 8, 2
SLOTS = M + 1

_compiled = None


def _prep(inputs):
    """Host-side parameter preparation (numpy, float64 internally)."""
    log_dt = inputs["log_dt"].astype(np.float64)
    A = -np.exp(inputs["log_A_real"].astype(np.float64)) \
        + 1j * inputs["A_imag"].astype(np.float64)
    dt = np.exp(log_dt)
    r = np.exp(dt[:, None] * A)                                   # (C, N)
    Bc = inputs["B_re"].astype(np.float64) + 1j * inputs["B_im"].astype(np.float64)
    Cc = inputs["C_re"].astype(np.float64) + 1j * inputs["C_im"].astype(np.float64)
    w = Cc * (r - 1.0) / A * Bc                                   # (C, N)
    rinv = 1.0 / r
    wL = w * r ** (L - 1)

    lags = np.arange(T)
    keff = np.real(wL[:, :, None] * rinv[:, :, None] ** lags).sum(1)  # (C, T)
    keff[:, 0] += inputs["D"].astype(np.float64)

    toep = np.zeros((C, T, T), np.float16)
    for d in range(T):
        idx = np.arange(T - d)
        toep[:, idx, idx + d] = keff[:, d].astype(np.float16)[:, None]

    pw = r[:, :, None] ** lags                                     # (C, N, T)
    v2 = np.concatenate([pw.real, pw.imag], 1).transpose(0, 2, 1)  # (C, T, 16)

    pw1 = w[:, :, None] * rinv[:, :, None] ** lags                 # (C, N, T)
    v1 = np.concatenate([pw1.real, -pw1.imag], 1)                  # (C, 16, T)

    e = r ** T
    s_idx = np.arange(SLOTS)
    Epow = e[:, :, None] ** np.maximum(s_idx - 1, 0)               # (C, N, S)
    Epow[:, :, 0] = 0.0
    Kpow = r[:, :, None] ** (L - 1 - T * s_idx)                    # (C, N, S)
    Kpow[:, :, 0] = 0.0
    Kpow[:, :, M:] = 0.0  # slot 32 unused

    # scale tiles [128, 16, 8, 2, 33]: partition p = 32q + ss;
    # ss<8 -> re-row n=ss, 8<=ss<16 -> im-row n=ss-8, ss>=16 junk (0).
    # channel c = q + 4j + 32g.
    def scale_tiles(Z):
        t1 = np.zeros((128, 16, 8, 2, SLOTS), np.float16)
        t2 = np.zeros((128, 16, 8, 2, SLOTS), np.float16)
        for q in range(4):
            for ss in range(16):
                p = 32 * q + ss
                n = ss % 8
                cs = q + 4 * np.arange(8)[None, :] + 32 * np.arange(16)[:, None]
                zr = Z[cs, n, :].real.astype(np.float16)           # (16g, 8j, S)
                zi = Z[cs, n, :].imag.astype(np.float16)
                t1[p, :, :, 0, :] = zr; t1[p, :, :, 1, :] = zr
                sgn = -1.0 if ss < 8 else 1.0
                t2[p, :, :, 0, :] = sgn * zi; t2[p, :, :, 1, :] = sgn * zi
        return t1, t2

    e1, e2 = scale_tiles(Epow)
    k1, k2 = scale_tiles(Kpow)

    wmix = np.ascontiguousarray(inputs["W_out"].T).astype(np.float16)   # (C, 1024)
    b_out = inputs["b_out"].astype(np.float32)
    bouta = np.ascontiguousarray(b_out[:512].reshape(4, 128).T)         # (128, 4)
    boutg = np.ascontiguousarray(b_out[512:].reshape(4, 128).T)
    wd = (inputs["W_dec"][0].astype(np.float32) / L).reshape(4, 128).T  # (128, 4)
    wdec = np.ascontiguousarray(np.repeat(wd[:, None, :], 2, axis=1))   # (128, 2, 4)
    bdec = inputs["b_dec"].astype(np.float32).reshape(1, 1)

    # B2 batched-load layouts: per 16-channel block
    # toep2[blk, i, jc, j] = toep[16 blk + jc, i, j]
    toep2 = np.ascontiguousarray(
        toep.reshape(32, 16, T, T).transpose(0, 2, 1, 3))
    # v1two[blk, 32 q + s, col, j]: q<3 channels at col h (c = 16blk+q+4h),
    # q=3 channels mirrored to base 0, cols 4+h.
    v1f = v1.astype(np.float16)
    v1two = np.zeros((32, 128, 8, T), np.float16)
    for blk in range(32):
        for jy in range(16):
            c = 16 * blk + jy
            q, h = jy % 4, jy // 4
            if q < 3:
                v1two[blk, 32 * q:32 * q + 16, h, :] = v1f[c]
            else:
                v1two[blk, 0:16, 4 + h, :] = v1f[c]

    aux = dict(
        toep2=toep2,
        v1two=v1two,
        v2m=np.ascontiguousarray(v2.astype(np.float16)),
        e1=e1, e2=e2, k1=k1, k2=k2,
        wmix=wmix, bouta=bouta, boutg=boutg, wdec=wdec, bdec=bdec,
    )
    u16 = inputs["u"].astype(np.float16)                                # (B, C, L)
    return aux, u16


def _build():
    nc = bacc.Bacc("TRN2", target_bir_lowering=False, debug=False,
                   num_devices=NCORES)
    d_u = nc.dram_tensor("u16", [BL, C, L], F16, kind="ExternalInput").ap()
    d_toep2 = nc.dram_tensor("toep2", [32, T, 16, T], F16, kind="ExternalInput").ap()
    d_v2 = nc.dram_tensor("v2m", [C, T, 16], F16, kind="ExternalInput").ap()
    d_v1two = nc.dram_tensor("v1two", [32, 128, 8, T], F16, kind="ExternalInput").ap()
    d_e1 = nc.dram_tensor("e1", [128, 16, 8, 2, SLOTS], F16, kind="ExternalInput").ap()
    d_e2 = nc.dram_tensor("e2", [128, 16, 8, 2, SLOTS], F16, kind="ExternalInput").ap()
    d_k1 = nc.dram_tensor("k1", [128, 16, 8, 2, SLOTS], F16, kind="ExternalInput").ap()
    d_k2 = nc.dram_tensor("k2", [128, 16, 8, 2, SLOTS], F16, kind="ExternalInput").ap()
    d_wmix = nc.dram_tensor("wmix", [C, 1024], F16, kind="ExternalInput").ap()
    d_bouta = nc.dram_tensor("bouta", [128, 4], F32, kind="ExternalInput").ap()
    d_boutg = nc.dram_tensor("boutg", [128, 4], F32, kind="ExternalInput").ap()
    d_wdec = nc.dram_tensor("wdec", [128, 2, 4], F32, kind="ExternalInput").ap()
    d_bdec = nc.dram_tensor("bdec", [1, 1], F32, kind="ExternalInput").ap()
    d_out = nc.dram_tensor("odec", [1, 2], F32, kind="ExternalOutput").ap()

    SHUF = [(i + 8) % 16 if i < 16 else i for i in range(32)]

    # DRAM staging for batch-1 gelu output (batch-0 stays in SBUF)
    d_stg = nc.dram_tensor("stg1", [4, 128, M, 128], F16, kind="Internal").ap()

    with tile.TileContext(nc) as tc:
        with tc.tile_pool(name="const", bufs=1) as constp, \
             tc.tile_pool(name="ytp", bufs=1) as ytp, \
             tc.tile_pool(name="wmx", bufs=1) as wmxp:
            ytF = {}

            # wmix preloaded on the (idle) gpsimd SWDGE queue
            wm = []
            for ctt in range(4):
                wt = wmxp.tile([128, 1024], F16,
                               name=f"wm{ctt}", tag=f"wm{ctt}")
                nc.gpsimd.dma_start(
                    wt[:], d_wmix[128 * ctt:128 * ctt + 128, :])
                wm.append(wt)

            bouta_sb = constp.tile([128, 4], F32)
            nc.sync.dma_start(bouta_sb[:], d_bouta[:])
            boutg_sb = constp.tile([128, 4], F32)
            nc.sync.dma_start(boutg_sb[:], d_boutg[:])
            wdec_sb = constp.tile([128, 2, 4], F32)
            nc.sync.dma_start(wdec_sb[:], d_wdec[:])
            bdec_sb = constp.tile([1, 1], F32)
            nc.sync.dma_start(bdec_sb[:], d_bdec[:])

            # ================= SSM phase ==================================
            with tc.tile_pool(name="uTp", bufs=1) as uTp, \
                 tc.tile_pool(name="Hp", bufs=1) as Hp, \
                 tc.tile_pool(name="lateb1", bufs=1) as latep:
                # per-ct tiles so B1 g-groups start after only their own
                # transposes land; round-robin 2 DMA queues
                uT = [uTp.tile([128, BL, M, 128], F16,
                               name=f"uT{i}", tag=f"uT{i}")
                      for i in range(4)]
                H = [Hp.tile([128, 8, 8, BL, SLOTS], F16,
                             name=f"H{i}", tag=f"H{i}")
                     for i in range(2)]
                H96 = [Hp.tile([32, 8, 8, BL, SLOTS], F16,
                               name=f"H96_{i}", tag=f"H96_{i}")
                       for i in range(2)]

                def u_rhs(c):
                    ct, cc = c // 128, c % 128
                    return uT[ct][:, :, :, cc:cc + 1].squeeze()

                # ---- Phase A: transpose u to time-major (DRAM source) ----
                for ct in range(4):
                    for b in range(BL):
                        nc.sync.dma_start_transpose(
                            uT[ct][:, b],
                            d_u[b, 128 * ct:128 * ct + 128, :])

                # ---- Phase B1: V2 matmuls + inline per-g scan ----
                nc.vector.memset(H[0][:, :, :, :, 0:1], 0.0)
                nc.vector.memset(H[1][:, :, :, :, 0:1], 0.0)
                with tc.tile_pool(name="v2p", bufs=3) as v2p, \
                     tc.tile_pool(name="esclp", bufs=6) as esclp, \
                     tc.tile_pool(name="sclscratch", bufs=1) as sclp, \
                     tc.tile_pool(name="hps", bufs=4, space="PSUM") as hps:
                    for g in range(16):
                        v2g = v2p.tile([128, 32, 16], F16)
                        nc.scalar.dma_start(
                            v2g[:], d_v2[32 * g:32 * g + 32].transpose([1, 0, 2]))
                        hb = hps.tile([128, 8, BL, M], F32)
                        for lc in range(32):
                            c = 32 * g + lc
                            q, jj = c % 4, (c // 4) % 8
                            nc.tensor.matmul(
                                hb[32 * q:32 * q + 16,
                                   jj:jj + 1, :, :].squeeze(),
                                v2g[:, lc, :], u_rhs(c),
                                start=True, stop=True,
                                tile_position=(0, 32 * q))
                        # inline prescale: H[g, slots 1..32] = E * hb (complex)
                        e1g = esclp.tile([128, 8, BL, M], F16, tag="e1")
                        nc.scalar.dma_start(e1g[:], d_e1[:, g, :, :, 1:SLOTS])
                        e2g = esclp.tile([128, 8, BL, M], F16, tag="e2")
                        nc.scalar.dma_start(e2g[:], d_e2[:, g, :, :, 1:SLOTS])
                        sw = sclp.tile([128, 8, BL, M], F32, tag="sw")
                        t1 = sclp.tile([128, 8, BL, M], F16, tag="t1")
                        hh, gg = g // 8, g % 8
                        nc.vector.stream_shuffle(sw[:], hb[:], SHUF)
                        nc.vector.tensor_mul(t1[:], hb[:], e1g[:])
                        nc.vector.tensor_mul(sw[:], sw[:], e2g[:])
                        nc.vector.tensor_add(
                            H[hh][:, gg:gg + 1, :, :, 1:SLOTS].squeeze(),
                            t1[:], sw[:])

                        if g % 8 == 7:
                            g0 = g - 7
                            for sl in range(2, M):
                                nc.vector.tensor_add(
                                    H[hh][:, :, :, :, sl:sl + 1],
                                    H[hh][:, :, :, :, sl:sl + 1],
                                    H[hh][:, :, :, :, sl - 1:sl])
                            # postscale: H = K * H (complex)
                            hgc = H[hh][:]
                            k1g = latep.tile([128, 8, 8, BL, SLOTS], F16, tag="k1")
                            nc.scalar.dma_start(k1g[:], d_k1[:, g0:g0 + 8])
                            k2g = latep.tile([128, 8, 8, BL, SLOTS], F16, tag="k2")
                            nc.scalar.dma_start(k2g[:], d_k2[:, g0:g0 + 8])
                            sw2 = latep.tile([128, 8, 8, BL, SLOTS], F16, tag="sw2")
                            t2 = latep.tile([128, 8, 8, BL, SLOTS], F16, tag="t2")
                            nc.vector.stream_shuffle(sw2[:], hgc, SHUF)
                            nc.vector.tensor_mul(t2[:], hgc, k1g[:])
                            nc.vector.tensor_mul(sw2[:], sw2[:], k2g[:])
                            nc.vector.tensor_add(hgc, t2[:], sw2[:])
                            nc.vector.tensor_copy(
                                H96[hh][0:16, :], H[hh][96:112, :])

                # ---- Phase B2: intra Toeplitz + V1 expand + GELU ----------
                with tc.tile_pool(name="tpp", bufs=2) as tpp, \
                     tc.tile_pool(name="v1p", bufs=2) as v1p, \
                     tc.tile_pool(name="stg", bufs=3) as stgp, \
                     tc.tile_pool(name="stct", bufs=1) as stctp, \
                     tc.tile_pool(name="yps", bufs=4, space="PSUM") as ypsp:
                    st2ct = None
                    for blk in range(32):
                        c0 = 16 * blk
                        if blk % 8 == 0:
                            st2ct = stctp.tile([128, BL, M, 128], F16)
                        toepb = tpp.tile([128, 16, T], F16)
                        nc.scalar.dma_start(toepb[:], d_toep2[blk])
                        v1b = v1p.tile([128, 8, T], F16)
                        nc.scalar.dma_start(v1b[:], d_v1two[blk])
                        yb = ypsp.tile([128, 16, BL, M], F32)   # 2 banks
                        for jy in range(16):
                            c = c0 + jy
                            q, h = jy % 4, jy // 4
                            g, jj = c // 32, (c // 4) % 8
                            hh, gg = g // 8, g % 8
                            nc.tensor.matmul(
                                yb[:, jy:jy + 1, :, :].squeeze(),
                                toepb[:, jy, :], u_rhs(c),
                                start=True, stop=False)
                            if q < 3:
                                h_rhs = H[hh][32 * q:32 * q + 16,
                                             gg:gg + 1, jj:jj + 1, :,
                                             0:M].squeeze()
                                v1_lhs = v1b[32 * q:32 * q + 16, h, :]
                            else:
                                h_rhs = H96[hh][0:16,
                                               gg:gg + 1, jj:jj + 1, :,
                                               0:M].squeeze()
                                v1_lhs = v1b[0:16, 4 + h, :]
                            nc.tensor.matmul(
                                yb[:, jy:jy + 1, :, :].squeeze(),
                                v1_lhs, h_rhs,
                                start=False, stop=True)
                        # GELU (contiguous dest) then DVE reorder
                        st1 = stgp.tile([128, 16, BL, M], F16, tag="st1")
                        nc.scalar.activation(st1[:], yb[:], AF.Gelu)
                        ct, cc = c0 // 128, c0 % 128
                        nc.vector.tensor_copy(
                            st2ct[:, :, :, cc:cc + 16].transpose([0, 3, 1, 2]),
                            st1[:])
                        if blk % 8 == 7:
                            # b=0: SBUF->SBUF XBAR transpose straight into the
                            # mix phase's channel-major tile (no DRAM trip);
                            # b=1: stage to DRAM (SBUF too small for both)
                            t = ytp.tile([128, M, 128], F16,
                                         name=f"yt0_{ct}", tag=f"yt0_{ct}")
                            nc.sync.dma_start_transpose(t[:], st2ct[:, 0])
                            ytF[(0, ct)] = t
                            nc.scalar.dma_start(d_stg[ct], st2ct[:, 1])

            # ================= Mix phase ==================================
            with tc.tile_pool(name="ytm", bufs=1) as ytm, \
                 tc.tile_pool(name="sgp", bufs=4) as sgp, \
                 tc.tile_pool(name="m1p", bufs=1) as m1p:
                # transpose-load the staged b=1 tiles (scalar queue orders
                # these after the d_stg stores issued at the end of B2)
                for ctt in range(4):
                    t = ytm.tile([128, M, 128], F16,
                                 name=f"yt1_{ctt}", tag=f"yt1_{ctt}")
                    nc.scalar.dma_start_transpose(t[:], d_stg[ctt])
                    ytF[(1, ctt)] = t
                M1 = m1p.tile([128, BL, 4, 8], F32)
                nts = [(b, lc) for b in range(BL) for lc in range(8)]
                with tc.tile_pool(name="zps", bufs=2, space="PSUM") as zpsp:
                    for pp in range(8):
                        for pr in range(4):
                            pair = nts[2 * pp:2 * pp + 2]
                            za2 = zpsp.tile([128, 2, 512], F32)
                            zg2 = zpsp.tile([128, 2, 512], F32)
                            for side, ztile in ((0, za2), (1, zg2)):
                                ot = pr + 4 * side
                                for ih, (b, lc) in enumerate(pair):
                                    for ctt in range(4):
                                        nc.tensor.matmul(
                                            ztile[:, ih:ih + 1, :].squeeze(),
                                            wm[ctt][:, 128 * ot:128 * ot + 128],
                                            ytF[(b, ctt)][:, 4 * lc:4 * lc + 4, :]
                                            .rearrange("p a b -> p (a b)"),
                                            start=(ctt == 0), stop=(ctt == 3))
                            sg2 = sgp.tile([128, 2, 512], F16, tag="sg")
                            nc.scalar.activation(
                                sg2[:], zg2[:], AF.Sigmoid,
                                bias=boutg_sb[:, pr:pr + 1])
                            for ih, (b, lc) in enumerate(pair):
                                scr = sgp.tile([128, 512], F16, tag="scr")
                                nc.vector.scalar_tensor_tensor(
                                    scr[:], za2[:, ih:ih + 1, :].squeeze(),
                                    bouta_sb[:, pr:pr + 1],
                                    sg2[:, ih:ih + 1, :].squeeze(),
                                    op0=ALU.add, op1=ALU.mult,
                                    accum_out=M1[:, b:b + 1, pr:pr + 1,
                                                 lc:lc + 1].squeeze().unsqueeze(1))

                # ---- decode ----
                with tc.tile_pool(name="dps", bufs=1, space="PSUM") as dpsp:
                    R1 = m1p.tile([128, BL, 4], F32)
                    nc.vector.reduce_sum(R1[:], M1[:], axis=mybir.AxisListType.X)
                    R2 = m1p.tile([128, BL, 4], F32)
                    nc.vector.tensor_mul(R2[:], R1[:], wdec_sb[:])
                    R3 = m1p.tile([128, BL], F32)
                    nc.vector.reduce_sum(R3[:], R2[:], axis=mybir.AxisListType.X)
                    ones = m1p.tile([128, 1], F32)
                    nc.vector.memset(ones[:], 1.0)
                    dp = dpsp.tile([1, 2], F32)
                    nc.tensor.matmul(dp[:], ones[:], R3[:], start=True, stop=True)
                    osb = m1p.tile([1, 2], F32)
                    nc.vector.tensor_scalar_add(osb[:], dp[:], bdec_sb[:, 0:1])
                    nc.sync.dma_start(d_out[:], osb[:])

    nc.compile()
    return nc


def _get_compiled():
    global _compiled
    if _compiled is None:
        _compiled = _build()
    return _compiled


def _run(inputs, trace=False, **kw):
    aux, u16 = _prep(inputs)
    nc = _get_compiled()
    in_maps = []
    for cid in range(NCORES):
        m = dict(aux)
        m["u16"] = np.ascontiguousarray(u16[BL * cid:BL * cid + BL])
        in_maps.append(m)
    return bass_utils.run_bass_kernel_spmd(
        nc, in_maps, core_ids=list(range(NCORES)), trace=trace, **kw)


def kernel(**inputs):
    inputs = {k: np.asarray(v) for k, v in inputs.items()}
    res = _run(inputs)
    out = np.empty((B, 1), np.float32)
    for cid in range(NCORES):
        out[BL * cid:BL * cid + BL, 0] = res.results[cid]["odec"][0, :]
    return out



# revision 33
# speedup vs baseline: 1.0971x; 1.0971x over previous
"""MiniS4D Trainium2 kernel (8 NeuronCores, batch-data-parallel).

Math: the reference convolves u with the TIME-REVERSED S4D kernel
(keff[lag] = K[L-1-lag], built from growing exponentials 1/r). Stable
chunked decomposition used here (T=128, M=32 chunks):

  y[l] = intra + inter
  intra (lags 0..127): per-channel Toeplitz matmul with keff[0:128]
         (+ D folded into lag 0).
  inter (lags >= 128): y_inter[128m+j] = Re sum_n w_n r_n^{-j} *
         Htil_n[m],  Htil[m] = r^{L-1-128m} * PP[m-1],
         PP[m] = prefix-sum over chunks of e^m * Q[m],  e = r^128,
         Q[m] = sum_i r^i u[128m+i]   (all factors bounded, |r|<1).

Per core: 2 batch elements, all 512 channels; fp16 operands, fp32
accumulation. Output (16,1) assembled on host from per-core (1,2).
"""
import sys, os
sys.path.insert(0, "/opt/trn_rl_repo")
import numpy as np

import concourse.bass as bass
import concourse.tile as tile
from concourse import bacc, mybir
from concourse import bass_utils

F32 = mybir.dt.float32
F16 = mybir.dt.float16
AF = mybir.ActivationFunctionType
ALU = mybir.AluOpType

B, C, L, N = 16, 512, 4096, 8
T, M = 128, 32
NCORES, BL = 8, 2
SLOTS = M + 1

_compiled = None


def _prep(inputs):
    """Host-side parameter preparation (numpy, float64 internally)."""
    log_dt = inputs["log_dt"].astype(np.float64)
    A = -np.exp(inputs["log_A_real"].astype(np.float64)) \
        + 1j * inputs["A_imag"].astype(np.float64)
    dt = np.exp(log_dt)
    r = np.exp(dt[:, None] * A)                                   # (C, N)
    Bc = inputs["B_re"].astype(np.float64) + 1j * inputs["B_im"].astype(np.float64)
    Cc = inputs["C_re"].astype(np.float64) + 1j * inputs["C_im"].astype(np.float64)
    w = Cc * (r - 1.0) / A * Bc                                   # (C, N)
    rinv = 1.0 / r
    wL = w * r ** (L - 1)

    lags = np.arange(T)
    keff = np.real(wL[:, :, None] * rinv[:, :, None] ** lags).sum(1)  # (C, T)
    keff[:, 0] += inputs["D"].astype(np.float64)

    toep = np.zeros((C, T, T), np.float16)
    for d in range(T):
        idx = np.arange(T - d)
        toep[:, idx, idx + d] = keff[:, d].astype(np.float16)[:, None]

    pw = r[:, :, None] ** lags                                     # (C, N, T)
    v2 = np.concatenate([pw.real, pw.imag], 1).transpose(0, 2, 1)  # (C, T, 16)

    pw1 = w[:, :, None] * rinv[:, :, None] ** lags                 # (C, N, T)
    v1 = np.concatenate([pw1.real, -pw1.imag], 1)                  # (C, 16, T)

    e = r ** T
    s_idx = np.arange(SLOTS)
    Epow = e[:, :, None] ** np.maximum(s_idx - 1, 0)               # (C, N, S)
    Epow[:, :, 0] = 0.0
    Kpow = r[:, :, None] ** (L - 1 - T * s_idx)                    # (C, N, S)
    Kpow[:, :, 0] = 0.0
    Kpow[:, :, M:] = 0.0  # slot 32 unused

    # scale tiles [128, 16, 8, 2, 33]: partition p = 32q + ss;
    # ss<8 -> re-row n=ss, 8<=ss<16 -> im-row n=ss-8, ss>=16 junk (0).
    # channel c = q + 4j + 32g.
    def scale_tiles(Z):
        t1 = np.zeros((128, 16, 8, 2, SLOTS), np.float16)
        t2 = np.zeros((128, 16, 8, 2, SLOTS), np.float16)
        for q in range(4):
            for ss in range(16):
                p = 32 * q + ss
                n = ss % 8
                cs = q + 4 * np.arange(8)[None, :] + 32 * np.arange(16)[:, None]
                zr = Z[cs, n, :].real.astype(np.float16)           # (16g, 8j, S)
                zi = Z[cs, n, :].imag.astype(np.float16)
                t1[p, :, :, 0, :] = zr; t1[p, :, :, 1, :] = zr
                sgn = -1.0 if ss < 8 else 1.0
                t2[p, :, :, 0, :] = sgn * zi; t2[p, :, :, 1, :] = sgn * zi
        return t1, t2

    e1, e2 = scale_tiles(Epow)
    k1, k2 = scale_tiles(Kpow)

    wmix = np.ascontiguousarray(inputs["W_out"].T).astype(np.float16)   # (C, 1024)
    b_out = inputs["b_out"].astype(np.float32)
    bouta = np.ascontiguousarray(b_out[:512].reshape(4, 128).T)         # (128, 4)
    boutg = np.ascontiguousarray(b_out[512:].reshape(4, 128).T)
    wd = (inputs["W_dec"][0].astype(np.float32) / L).reshape(4, 128).T  # (128, 4)
    wdec = np.ascontiguousarray(np.repeat(wd[:, None, :], 2, axis=1))   # (128, 2, 4)
    bdec = inputs["b_dec"].astype(np.float32).reshape(1, 1)

    # B2 batched-load layouts: per 16-channel block
    # toep2[blk, i, jc, j] = toep[16 blk + jc, i, j]
    toep2 = np.ascontiguousarray(
        toep.reshape(32, 16, T, T).transpose(0, 2, 1, 3))
    # v1two[blk, 32 q + s, col, j]: q<3 channels at col h (c = 16blk+q+4h),
    # q=3 channels mirrored to base 0, cols 4+h.
    v1f = v1.astype(np.float16)
    v1two = np.zeros((32, 128, 8, T), np.float16)
    for blk in range(32):
        for jy in range(16):
            c = 16 * blk + jy
            q, h = jy % 4, jy // 4
            if q < 3:
                v1two[blk, 32 * q:32 * q + 16, h, :] = v1f[c]
            else:
                v1two[blk, 0:16, 4 + h, :] = v1f[c]

    # per-g contiguous DRAM layouts (strided gathers explode into 32B
    # DMA descriptors and stall B1 for ~15us per load)
    v2m = np.ascontiguousarray(
        v2.astype(np.float16).reshape(16, 32, T, 16).transpose(0, 2, 1, 3))
    e1n = np.ascontiguousarray(e1[:, :, :, :, 1:].transpose(1, 0, 2, 3, 4))
    e2n = np.ascontiguousarray(e2[:, :, :, :, 1:].transpose(1, 0, 2, 3, 4))
    aux = dict(
        toep2=toep2,
        v1two=v1two,
        v2m=v2m,
        e1=e1n, e2=e2n, k1=k1, k2=k2,
        wmix=wmix, bouta=bouta, boutg=boutg, wdec=wdec, bdec=bdec,
    )
    u16 = inputs["u"].astype(np.float16)                                # (B, C, L)
    return aux, u16


def _build():
    nc = bacc.Bacc("TRN2", target_bir_lowering=False, debug=False,
                   num_devices=NCORES)
    d_u = nc.dram_tensor("u16", [BL, C, L], F16, kind="ExternalInput").ap()
    d_toep2 = nc.dram_tensor("toep2", [32, T, 16, T], F16, kind="ExternalInput").ap()
    d_v2 = nc.dram_tensor("v2m", [16, T, 32, 16], F16, kind="ExternalInput").ap()
    d_v1two = nc.dram_tensor("v1two", [32, 128, 8, T], F16, kind="ExternalInput").ap()
    d_e1 = nc.dram_tensor("e1", [16, 128, 8, 2, M], F16, kind="ExternalInput").ap()
    d_e2 = nc.dram_tensor("e2", [16, 128, 8, 2, M], F16, kind="ExternalInput").ap()
    d_k1 = nc.dram_tensor("k1", [128, 16, 8, 2, SLOTS], F16, kind="ExternalInput").ap()
    d_k2 = nc.dram_tensor("k2", [128, 16, 8, 2, SLOTS], F16, kind="ExternalInput").ap()
    d_wmix = nc.dram_tensor("wmix", [C, 1024], F16, kind="ExternalInput").ap()
    d_bouta = nc.dram_tensor("bouta", [128, 4], F32, kind="ExternalInput").ap()
    d_boutg = nc.dram_tensor("boutg", [128, 4], F32, kind="ExternalInput").ap()
    d_wdec = nc.dram_tensor("wdec", [128, 2, 4], F32, kind="ExternalInput").ap()
    d_bdec = nc.dram_tensor("bdec", [1, 1], F32, kind="ExternalInput").ap()
    d_out = nc.dram_tensor("odec", [1, 2], F32, kind="ExternalOutput").ap()

    SHUF = [(i + 8) % 16 if i < 16 else i for i in range(32)]

    # DRAM staging for batch-1 gelu output (batch-0 stays in SBUF)
    d_stg = nc.dram_tensor("stg1", [4, 128, M, 128], F16, kind="Internal").ap()

    with tile.TileContext(nc) as tc:
        with tc.tile_pool(name="const", bufs=1) as constp, \
             tc.tile_pool(name="ytp", bufs=1) as ytp, \
             tc.tile_pool(name="wmx", bufs=1) as wmxp:
            ytF = {}

            # wmix preloaded on the (idle) gpsimd SWDGE queue
            wm = []
            for ctt in range(4):
                wt = wmxp.tile([128, 1024], F16,
                               name=f"wm{ctt}", tag=f"wm{ctt}")
                nc.gpsimd.dma_start(
                    wt[:], d_wmix[128 * ctt:128 * ctt + 128, :])
                wm.append(wt)

            bouta_sb = constp.tile([128, 4], F32)
            nc.sync.dma_start(bouta_sb[:], d_bouta[:])
            boutg_sb = constp.tile([128, 4], F32)
            nc.sync.dma_start(boutg_sb[:], d_boutg[:])
            wdec_sb = constp.tile([128, 2, 4], F32)
            nc.sync.dma_start(wdec_sb[:], d_wdec[:])
            bdec_sb = constp.tile([1, 1], F32)
            nc.sync.dma_start(bdec_sb[:], d_bdec[:])

            # ================= SSM phase ==================================
            with tc.tile_pool(name="uTp", bufs=1) as uTp, \
                 tc.tile_pool(name="Hp", bufs=1) as Hp, \
                 tc.tile_pool(name="lateb1", bufs=1) as latep:
                # per-ct tiles so B1 g-groups start after only their own
                # transposes land; round-robin 2 DMA queues
                uT = [uTp.tile([128, BL, M, 128], F16,
                               name=f"uT{i}", tag=f"uT{i}")
                      for i in range(4)]
                H = [Hp.tile([128, 8, 8, BL, SLOTS], F16,
                             name=f"H{i}", tag=f"H{i}")
                     for i in range(2)]
                H96 = [Hp.tile([32, 8, 8, BL, SLOTS], F16,
                               name=f"H96_{i}", tag=f"H96_{i}")
                       for i in range(2)]

                def u_rhs(c):
                    ct, cc = c // 128, c % 128
                    return uT[ct][:, :, :, cc:cc + 1].squeeze()

                # ---- Phase A: transpose u to time-major (DRAM source) ----
                for ct in range(4):
                    for b in range(BL):
                        nc.sync.dma_start_transpose(
                            uT[ct][:, b],
                            d_u[b, 128 * ct:128 * ct + 128, :])

                # ---- Phase B1: V2 matmuls + inline per-g scan ----
                nc.vector.memset(H[0][:, :, :, :, 0:1], 0.0)
                nc.vector.memset(H[1][:, :, :, :, 0:1], 0.0)
                with tc.tile_pool(name="v2p", bufs=3) as v2p, \
                     tc.tile_pool(name="esclp", bufs=6) as esclp, \
                     tc.tile_pool(name="sclscratch", bufs=1) as sclp, \
                     tc.tile_pool(name="hps", bufs=4, space="PSUM") as hps:
                    for g in range(16):
                        v2g = v2p.tile([128, 32, 16], F16)
                        nc.scalar.dma_start(v2g[:], d_v2[g])
                        hb = hps.tile([128, 8, BL, M], F32)
                        for lc in range(32):
                            c = 32 * g + lc
                            q, jj = c % 4, (c // 4) % 8
                            nc.tensor.matmul(
                                hb[32 * q:32 * q + 16,
                                   jj:jj + 1, :, :].squeeze(),
                                v2g[:, lc, :], u_rhs(c),
                                start=True, stop=True,
                                tile_position=(0, 32 * q))
                        # inline prescale: H[g, slots 1..32] = E * hb (complex)
                        e1g = esclp.tile([128, 8, BL, M], F16, tag="e1")
                        nc.scalar.dma_start(e1g[:], d_e1[g])
                        e2g = esclp.tile([128, 8, BL, M], F16, tag="e2")
                        nc.scalar.dma_start(e2g[:], d_e2[g])
                        sw = sclp.tile([128, 8, BL, M], F32, tag="sw")
                        t1 = sclp.tile([128, 8, BL, M], F16, tag="t1")
                        hh, gg = g // 8, g % 8
                        nc.vector.stream_shuffle(sw[:], hb[:], SHUF)
                        nc.vector.tensor_mul(t1[:], hb[:], e1g[:])
                        nc.vector.tensor_mul(sw[:], sw[:], e2g[:])
                        nc.vector.tensor_add(
                            H[hh][:, gg:gg + 1, :, :, 1:SLOTS].squeeze(),
                            t1[:], sw[:])

                        if g % 8 == 7:
                            g0 = g - 7
                            for sl in range(2, M):
                                nc.vector.tensor_add(
                                    H[hh][:, :, :, :, sl:sl + 1],
                                    H[hh][:, :, :, :, sl:sl + 1],
                                    H[hh][:, :, :, :, sl - 1:sl])
                            # postscale: H = K * H (complex)
                            hgc = H[hh][:]
                            k1g = latep.tile([128, 8, 8, BL, SLOTS], F16, tag="k1")
                            nc.scalar.dma_start(k1g[:], d_k1[:, g0:g0 + 8])
                            k2g = latep.tile([128, 8, 8, BL, SLOTS], F16, tag="k2")
                            nc.scalar.dma_start(k2g[:], d_k2[:, g0:g0 + 8])
                            sw2 = latep.tile([128, 8, 8, BL, SLOTS], F16, tag="sw2")
                            t2 = latep.tile([128, 8, 8, BL, SLOTS], F16, tag="t2")
                            nc.vector.stream_shuffle(sw2[:], hgc, SHUF)
                            nc.vector.tensor_mul(t2[:], hgc, k1g[:])
                            nc.vector.tensor_mul(sw2[:], sw2[:], k2g[:])
                            nc.vector.tensor_add(hgc, t2[:], sw2[:])
                            nc.vector.tensor_copy(
                                H96[hh][0:16, :], H[hh][96:112, :])

                # ---- Phase B2: intra Toeplitz + V1 expand + GELU ----------
                with tc.tile_pool(name="tpp", bufs=2) as tpp, \
                     tc.tile_pool(name="v1p", bufs=2) as v1p, \
                     tc.tile_pool(name="stg", bufs=3) as stgp, \
                     tc.tile_pool(name="stct", bufs=1) as stctp, \
                     tc.tile_pool(name="yps", bufs=4, space="PSUM") as ypsp:
                    st2ct = None
                    for blk in range(32):
                        c0 = 16 * blk
                        if blk % 8 == 0:
                            st2ct = stctp.tile([128, BL, M, 128], F16)
                        toepb = tpp.tile([128, 16, T], F16)
                        nc.scalar.dma_start(toepb[:], d_toep2[blk])
                        v1b = v1p.tile([128, 8, T], F16)
                        nc.scalar.dma_start(v1b[:], d_v1two[blk])
                        yb = ypsp.tile([128, 16, BL, M], F32)   # 2 banks
                        for jy in range(16):
                            c = c0 + jy
                            q, h = jy % 4, jy // 4
                            g, jj = c // 32, (c // 4) % 8
                            hh, gg = g // 8, g % 8
                            nc.tensor.matmul(
                                yb[:, jy:jy + 1, :, :].squeeze(),
                                toepb[:, jy, :], u_rhs(c),
                                start=True, stop=False)
                            if q < 3:
                                h_rhs = H[hh][32 * q:32 * q + 16,
                                             gg:gg + 1, jj:jj + 1, :,
                                             0:M].squeeze()
                                v1_lhs = v1b[32 * q:32 * q + 16, h, :]
                            else:
                                h_rhs = H96[hh][0:16,
                                               gg:gg + 1, jj:jj + 1, :,
                                               0:M].squeeze()
                                v1_lhs = v1b[0:16, 4 + h, :]
                            nc.tensor.matmul(
                                yb[:, jy:jy + 1, :, :].squeeze(),
                                v1_lhs, h_rhs,
                                start=False, stop=True)
                        # GELU (contiguous dest) then DVE reorder
                        st1 = stgp.tile([128, 16, BL, M], F16, tag="st1")
                        nc.scalar.activation(st1[:], yb[:], AF.Gelu)
                        ct, cc = c0 // 128, c0 % 128
                        nc.vector.tensor_copy(
                            st2ct[:, :, :, cc:cc + 16].transpose([0, 3, 1, 2]),
                            st1[:])
                        if blk % 8 == 7:
                            # b=0: SBUF->SBUF XBAR transpose straight into the
                            # mix phase's channel-major tile (no DRAM trip);
                            # b=1: stage to DRAM (SBUF too small for both)
                            t = ytp.tile([128, M, 128], F16,
                                         name=f"yt0_{ct}", tag=f"yt0_{ct}")
                            nc.sync.dma_start_transpose(t[:], st2ct[:, 0])
                            ytF[(0, ct)] = t
                            nc.scalar.dma_start(d_stg[ct], st2ct[:, 1])

            # ================= Mix phase ==================================
            with tc.tile_pool(name="ytm", bufs=1) as ytm, \
                 tc.tile_pool(name="sgp", bufs=4) as sgp, \
                 tc.tile_pool(name="m1p", bufs=1) as m1p:
                # transpose-load the staged b=1 tiles (scalar queue orders
                # these after the d_stg stores issued at the end of B2)
                for ctt in range(4):
                    t = ytm.tile([128, M, 128], F16,
                                 name=f"yt1_{ctt}", tag=f"yt1_{ctt}")
                    nc.scalar.dma_start_transpose(t[:], d_stg[ctt])
                    ytF[(1, ctt)] = t
                M1 = m1p.tile([128, BL, 4, 8], F32)
                nts = [(b, lc) for b in range(BL) for lc in range(8)]
                with tc.tile_pool(name="zps", bufs=2, space="PSUM") as zpsp:
                    for pp in range(8):
                        for pr in range(4):
                            pair = nts[2 * pp:2 * pp + 2]
                            za2 = zpsp.tile([128, 2, 512], F32)
                            zg2 = zpsp.tile([128, 2, 512], F32)
                            for side, ztile in ((0, za2), (1, zg2)):
                                ot = pr + 4 * side
                                for ih, (b, lc) in enumerate(pair):
                                    for ctt in range(4):
                                        nc.tensor.matmul(
                                            ztile[:, ih:ih + 1, :].squeeze(),
                                            wm[ctt][:, 128 * ot:128 * ot + 128],
                                            ytF[(b, ctt)][:, 4 * lc:4 * lc + 4, :]
                                            .rearrange("p a b -> p (a b)"),
                                            start=(ctt == 0), stop=(ctt == 3))
                            sg2 = sgp.tile([128, 2, 512], F16, tag="sg")
                            nc.scalar.activation(
                                sg2[:], zg2[:], AF.Sigmoid,
                                bias=boutg_sb[:, pr:pr + 1])
                            for ih, (b, lc) in enumerate(pair):
                                scr = sgp.tile([128, 512], F16, tag="scr")
                                nc.vector.scalar_tensor_tensor(
                                    scr[:], za2[:, ih:ih + 1, :].squeeze(),
                                    bouta_sb[:, pr:pr + 1],
                                    sg2[:, ih:ih + 1, :].squeeze(),
                                    op0=ALU.add, op1=ALU.mult,
                                    accum_out=M1[:, b:b + 1, pr:pr + 1,
                                                 lc:lc + 1].squeeze().unsqueeze(1))

                # ---- decode ----
                with tc.tile_pool(name="dps", bufs=1, space="PSUM") as dpsp:
                    R1 = m1p.tile([128, BL, 4], F32)
                    nc.vector.reduce_sum(R1[:], M1[:], axis=mybir.AxisListType.X)
                    R2 = m1p.tile([128, BL, 4], F32)
                    nc.vector.tensor_mul(R2[:], R1[:], wdec_sb[:])
                    R3 = m1p.tile([128, BL], F32)
                    nc.vector.reduce_sum(R3[:], R2[:], axis=mybir.AxisListType.X)
                    ones = m1p.tile([128, 1], F32)
                    nc.vector.memset(ones[:], 1.0)
                    dp = dpsp.tile([1, 2], F32)
                    nc.tensor.matmul(dp[:], ones[:], R3[:], start=True, stop=True)
                    osb = m1p.tile([1, 2], F32)
                    nc.vector.tensor_scalar_add(osb[:], dp[:], bdec_sb[:, 0:1])
                    nc.sync.dma_start(d_out[:], osb[:])

    nc.compile()
    return nc


def _get_compiled():
    global _compiled
    if _compiled is None:
        _compiled = _build()
    return _compiled


def _run(inputs, trace=False, **kw):
    aux, u16 = _prep(inputs)
    nc = _get_compiled()
    in_maps = []
    for cid in range(NCORES):
        m = dict(aux)
        m["u16"] = np.ascontiguousarray(u16[BL * cid:BL * cid + BL])
        in_maps.append(m)
    return bass_utils.run_bass_kernel_spmd(
        nc, in_maps, core_ids=list(range(NCORES)), trace=trace, **kw)


def kernel(**inputs):
    inputs = {k: np.asarray(v) for k, v in inputs.items()}
    res = _run(inputs)
    out = np.empty((B, 1), np.float32)
    for cid in range(NCORES):
        out[BL * cid:BL * cid + BL, 0] = res.results[cid]["odec"][0, :]
    return out

